# revision 26
# baseline (speedup 1.0000x reference)
"""Trainium2 Bass kernel for causal multi-head attention with RoPE.

Problem: B=4, S=2048, D=1024, H=16, DK=64 dense transformer attention
(q/k/v projections -> interleaved RoPE on q,k -> causal softmax attention
-> output projection), fp32 inputs/outputs.

Sharding: 8 NeuronCores, core c handles batch b=c//2 and head-group
g=c%2 (8 of the 16 heads).  Each core computes a partial o_proj output
for its batch over its heads; the host sums the two partials per batch.

Kernel design (per core) — v1 (mixed precision + pipelined):
  - q/k/v projections in fp8e4 with DoubleRow perf mode (2 contraction
    chunks per matmul, 0.5 cyc/row): host delivers x and Wq/Wk/Wv in
    fp8e4, kk-chunk-major layouts so DoubleRow pair APs are strided views.
  - RoPE in bf16 on DVE (2x/4x packed modes) using host cos/sin tables;
    results DMA-permuted into head-contiguous bf16 qrh/krh tiles.
  - scores in bf16: S_ps[kv, q] = k_chunk @ qT, two heads per PE pass via
    tile_position row groups (K=64 each), both heads' scores in one
    [128, 1024] PSUM tile (two banks); ONE merged exp per chunk
    (strided [128, 2, N] access) -> pt bf16.
  - v stored bf16 with a per-head ones-column (65 cols/head) so attn@v
    also produces the softmax denominator row; attn@v in bf16.
  - software pipelining: scores/exp of chunk c+1 issue before attn@v of
    chunk c, so the PE never waits on the Act engine exp.
  - phase interleave: attention for q-group qg runs between projection
    blocks of sc=qg+1, overlapping projection PE work with attention
    Act/DVE work.
  - normalize: broadcast the denominator row l across 64 partitions with
    a K=1 ones matmul into a shared scratch PSUM bank, reciprocal +
    multiply on DVE -> normalized bf16 outT per head.
  - o_proj in bf16 accumulated in PSUM; outputs DMA'd as bf16, host sums
    the two per-batch partials in fp32.
"""

import sys

sys.path.insert(0, "/opt/trn_rl_repo")

from contextlib import ExitStack

import numpy as np

import concourse.bass as bass
import concourse.tile as tile
from concourse import bacc, mybir
from concourse.bass_utils import run_bass_kernel_spmd

B, S, D, H = 4, 2048, 1024, 16
DK = D // H          # 64
NHL = 8              # heads per core (local)
QR = NHL * DK        # 512 projected rows per core
NKC = S // 128       # 16 kv chunks
THETA = 10000.0

F32 = mybir.dt.float32
BF16 = mybir.dt.bfloat16
F8 = mybir.dt.float8e4
DR = mybir.MatmulPerfMode.DoubleRow

_COMPILED = None


def build_kernel(reps=1):
    nc = bacc.Bacc("TRN2", target_bir_lowering=False, debug=False,
                   enable_asserts=False)

    x8d = nc.dram_tensor("x8", [128, 4 * 8 * 512], BF16, kind="ExternalInput").ap()
    wq8 = nc.dram_tensor("wq8", [128, 8 * QR], BF16, kind="ExternalInput").ap()
    wk8 = nc.dram_tensor("wk8", [128, 8 * QR], BF16, kind="ExternalInput").ap()
    wv8 = nc.dram_tensor("wv8", [128, 8 * QR], BF16, kind="ExternalInput").ap()
    wod = nc.dram_tensor("wod", [128, 4 * D], BF16, kind="ExternalInput").ap()
    cos4 = nc.dram_tensor("cos4", [128, S], BF16, kind="ExternalInput").ap()
    sin4 = nc.dram_tensor("sin4", [128, S], BF16, kind="ExternalInput").ap()
    maskd = nc.dram_tensor("maskd", [128, 256], BF16, kind="ExternalInput").ap()
    out = nc.dram_tensor("out", [S, D], BF16, kind="ExternalOutput").ap()

    with tile.TileContext(nc) as tc, ExitStack() as ctx:
        persist = ctx.enter_context(tc.tile_pool(name="persist", bufs=1))
        # head-contiguous rope'd q/k: chunk hp holds heads (2hp, 2hp+1);
        # within a head: [even-lane j 0..31 ; odd-lane j 0..31]
        qrh = [persist.tile([128, S], BF16, tag=f"qrh{i}", name=f"qrh{i}")
               for i in range(4)]
        krh = [persist.tile([128, S], BF16, tag=f"krh{i}", name=f"krh{i}")
               for i in range(4)]
        # v natural layout, 65 cols per head (64 v + ones), all 16 s-tiles
        v_all = persist.tile([128, NKC * NHL * 65], BF16, tag="v_all")
        vsb = [v_all[:, i * NHL * 65:(i + 1) * NHL * 65] for i in range(NKC)]
        maskt = persist.tile([128, 256], BF16, tag="maskt")
        onest = persist.tile([65, 64], BF16, tag="onest")
        cost_all = persist.tile([128, S], BF16, tag="cost")
        sint_all = persist.tile([128, S], BF16, tag="sint")
        wq = persist.tile([128, 8 * QR], BF16, tag="wq")
        wk = persist.tile([128, 8 * QR], BF16, tag="wk")
        wv = persist.tile([128, 8 * QR], BF16, tag="wv")
        woh = persist.tile([128, 4 * D], BF16, tag="woh")

        nc.sync.dma_start(wq[:], wq8[:])
        nc.sync.dma_start(cost_all[:], cos4[:])
        nc.sync.dma_start(sint_all[:], sin4[:])
        nc.sync.dma_start(wv[:], wv8[:])
        nc.sync.dma_start(wk[:], wk8[:])
        nc.scalar.dma_start(maskt[:], maskd[:])
        m3 = maskt[:].rearrange("p (two n) -> p two n", two=2)
        nc.scalar.dma_start(woh[:], wod[:])
        nc.vector.memset(onest[:], 1.0)
        # ones column (col 64 of each head's 65-col block), all kv tiles
        v3 = v_all[:].rearrange("p (n c) -> p n c", c=65)
        nc.gpsimd.memset(v3[:, :, 64:65], 1.0)

        wq3 = wq[:].rearrange("p (k q) -> p k q", k=8)
        wk3 = wk[:].rearrange("p (k q) -> p k q", k=8)
        wv3 = wv[:].rearrange("p (k q) -> p k q", k=8)

        xpool = ctx.enter_context(tc.tile_pool(name="xp", bufs=2))
        stg = ctx.enter_context(tc.tile_pool(name="stg", bufs=2))
        ppool = ctx.enter_context(tc.tile_pool(name="pt", bufs=6))
        otn = ctx.enter_context(tc.tile_pool(name="otn", bufs=6))
        # PSUM budget (8 banks): scratch 2 + sp 2x2 + O 2
        ps_x = ctx.enter_context(
            tc.tile_pool(name="ps_x", bufs=2, space="PSUM"))
        ps_s = ctx.enter_context(
            tc.tile_pool(name="ps_s", bufs=2, space="PSUM"))
        ps_o = ctx.enter_context(
            tc.tile_pool(name="ps_o", bufs=1, space="PSUM"))

        def project(w3, names):
            """q/k projection: 4 m-chunks of [128, 512], bf16."""
            sb = []
            for m in range(4):
                ps = ps_x.tile([128, 512], F32, tag="scr", name="pps")
                for t in range(8):
                    nc.tensor.matmul(
                        ps[:],
                        w3[:, t, m * 128:(m + 1) * 128],
                        xt3[:, t, :],
                        start=(t == 0), stop=(t == 7))
                qs = stg.tile([128, 512], BF16, tag="qps", bufs=8,
                              name=f"{names}{m}")
                nc.vector.tensor_copy(qs[:], ps[:])
                sb.append(qs)
            return sb

        def rope(sb, dst, s0, qk):
            # chunks (0,2) even/odd of heads 0-3, (1,3) heads 4-7
            costc = cost_all[:, s0:s0 + 512]
            sintc = sint_all[:, s0:s0 + 512]
            for me, mo in ((0, 2), (1, 3)):
                hbase = 0 if me == 0 else 4
                te = stg.tile([128, 512], BF16, tag="tmp", bufs=4)
                to = stg.tile([128, 512], BF16, tag="tmp", bufs=4)
                nc.vector.tensor_mul(te[:], sb[me][:], costc)
                nc.gpsimd.tensor_mul(to[:], sb[mo][:], sintc)
                qre = stg.tile([128, 512], BF16, tag="qr", bufs=4)
                nc.vector.tensor_sub(qre[:], te[:], to[:])
                te2 = stg.tile([128, 512], BF16, tag="tmp", bufs=4)
                to2 = stg.tile([128, 512], BF16, tag="tmp", bufs=4)
                nc.gpsimd.tensor_mul(te2[:], sb[mo][:], costc)
                nc.vector.tensor_mul(to2[:], sb[me][:], sintc)
                qro = stg.tile([128, 512], BF16, tag="qr", bufs=4)
                nc.vector.tensor_add(qro[:], te2[:], to2[:])
                # permute into head-contiguous chunks via DMA
                for hl in range(4):
                    h = hbase + hl
                    hp, h01 = h // 2, h % 2
                    nc.sync.dma_start(
                        dst[hp][64 * h01: 64 * h01 + 32, s0:s0 + 512],
                        qre[32 * hl: 32 * hl + 32, :])
                    nc.sync.dma_start(
                        dst[hp][64 * h01 + 32: 64 * h01 + 64, s0:s0 + 512],
                        qro[32 * hl: 32 * hl + 32, :])

        def attention(qg):
            q0 = qg * 512
            nchunks = 4 * qg + 4
            otn_tiles = [None] * 4
            for hp in range(4):
                O = [ps_o.tile([65, 512], F32, tag=f"O{h01}", name="O")
                     for h01 in range(2)]
                pend = None

                def emit_av(ent):
                    c, pt, qoff, N = ent
                    for h01 in range(2):
                        h = 2 * hp + h01
                        nc.tensor.matmul(
                            O[h01][:, qoff:qoff + N],
                            vsb[c][:, 65 * h: 65 * h + 65],
                            pt[:, 512 * h01: 512 * h01 + N],
                            start=(c == 0), stop=(c == nchunks - 1))

                for c in range(nchunks):
                    cmod = c - 4 * qg
                    qoff = 128 * cmod if cmod >= 0 else 0
                    N = 512 - qoff
                    sp = ps_s.tile([128, 1024], F32, tag="S")
                    for h01 in range(2):
                        base = 64 * h01
                        nc.tensor.matmul(
                            sp[:, 512 * h01: 512 * h01 + N],
                            krh[hp][base:base + 64, c * 128:(c + 1) * 128],
                            qrh[hp][base:base + 64, q0 + qoff:q0 + qoff + N],
                            start=True, stop=True,
                            tile_position=(base, 0))
                    pt = ppool.tile([128, 1024], BF16, tag="pt", bufs=6)
                    if N == 512:
                        nc.scalar.activation(
                            pt[:], sp[:],
                            mybir.ActivationFunctionType.Exp, scale=0.125)
                    else:
                        sp3 = sp[:].rearrange("p (two n) -> p two n", two=2)
                        pt3 = pt[:].rearrange("p (two n) -> p two n", two=2)
                        nc.scalar.activation(
                            pt3[:, :, 0:N], sp3[:, :, 0:N],
                            mybir.ActivationFunctionType.Exp, scale=0.125)
                    if cmod >= 0:
                        # causal mask: zero upper triangle post-exp
                        pt3 = pt[:].rearrange("p (two n) -> p two n", two=2)
                        nc.gpsimd.tensor_mul(pt3[:, :, 0:128],
                                             pt3[:, :, 0:128], m3[:])
                    if pend is not None:
                        emit_av(pend)
                    pend = (c, pt, qoff, N)
                emit_av(pend)

                pair = otn.tile([128, 512], BF16, tag="pair", bufs=6,
                                name="pair")
                for h01 in range(2):
                    lsb = stg.tile([65, 512], BF16, tag="lsb", bufs=2)
                    nc.vector.tensor_copy(lsb[64:65, :], O[h01][64:65, :])
                    rbp = ps_x.tile([128, 512], F32, tag="scr", name="rbp")
                    nc.tensor.matmul(rbp[0:64, :],
                                     onest[64:65, 0:64],
                                     lsb[64:65, :],
                                     start=True, stop=True)
                    rlb = stg.tile([64, 512], F32, tag="rlb", bufs=2)
                    nc.vector.reciprocal(rlb[:], rbp[0:64, :])
                    if h01 == 0:
                        nc.vector.tensor_mul(pair[0:64, :],
                                             O[h01][0:64, :], rlb[:])
                    else:
                        ot = stg.tile([64, 512], BF16, tag="ot", bufs=2)
                        nc.vector.tensor_mul(ot[:], O[h01][0:64, :],
                                             rlb[:])
                        nc.gpsimd.dma_start(pair[64:128, :], ot[:])
                otn_tiles[hp] = pair

            # o_proj for this q-group
            for qt in range(4):
                qtile = qg * 4 + qt
                osb = stg.tile([128, 1024], BF16, tag="osb", bufs=3)
                for oh in range(2):
                    f = ps_x.tile([128, 512], F32, tag="scr", name="F")
                    for p in range(4):
                        nc.tensor.matmul(
                            f[:],
                            otn_tiles[p][:, qt * 128:(qt + 1) * 128],
                            woh[:, p * D + oh * 512: p * D + oh * 512 + 512],
                            start=(p == 0), stop=(p == 3))
                    nc.vector.tensor_copy(osb[:, oh * 512:(oh + 1) * 512],
                                          f[:])
                nc.sync.dma_start(
                    out[qtile * 128:(qtile + 1) * 128, :], osb[:])

        for _rep in range(reps):
            xts = {}

            def load_xt(sc):
                xt = xpool.tile([128, 8 * 512], BF16, tag="xt")
                nc.sync.dma_start(xt[:], x8d[:, sc * 4096:(sc + 1) * 4096])
                xts[sc] = xt[:].rearrange("p (k s) -> p k s", k=8)

            load_xt(0)
            load_xt(1)
            for sc in range(4):
                s0 = sc * 512
                if sc > 0:
                    attention(sc - 1)
                xt3 = xts[sc]

                qsb = project(wq3, "q")
                if sc < 2:
                    load_xt(sc + 2)
                # v in natural layout (keeps PE busy while rope runs on DVE)
                for st in range(4):
                    vp = ps_x.tile([128, 512], F32, tag="scr", name="vp")
                    for t in range(8):
                        nc.tensor.matmul(
                            vp[:],
                            xt3[:, t, st * 128:(st + 1) * 128],
                            wv3[:, t, :],
                            start=(t == 0), stop=(t == 7))
                    vdst = vsb[sc * 4 + st][:].rearrange(
                        "p (h c) -> p h c", c=65)[:, :, 0:64]
                    vsrc = vp[:].rearrange("p (h c) -> p h c", c=64)
                    nc.vector.tensor_copy(vdst, vsrc)
                rope(qsb, qrh, s0, "q")
                ksb = project(wk3, "k")
                rope(ksb, krh, s0, "k")
            attention(3)

    nc.compile()
    return nc


def _rope_perm():
    """Row permutation for Wq/Wk per-core slices: 4 chunks of 128 =
    (heads 0-3 even, heads 4-7 even, heads 0-3 odd, heads 4-7 odd)."""
    perm = []
    for half in (0, 1):
        for hblk in range(2):
            for h in range(4 * hblk, 4 * hblk + 4):
                for j in range(32):
                    perm.append(h * 64 + 2 * j + half)
    return np.array(perm)


def _prep_in_maps(x, token_positions, Wq, Wk, Wv, Wo):
    BF = mybir.dt.np(BF16)
    half = DK // 2
    freqs = (1.0 / (THETA ** (2.0 * np.arange(half, dtype=np.float32) / DK)))
    angles = token_positions.astype(np.float32)[:, None] * freqs[None, :]
    cos = np.cos(angles).astype(np.float32).T    # [32, S]
    sin = np.sin(angles).astype(np.float32).T
    cos4 = np.ascontiguousarray(np.tile(cos, (4, 1))).astype(BF)  # [128, S]
    sin4 = np.ascontiguousarray(np.tile(sin, (4, 1))).astype(BF)

    kv_l = np.arange(128)[:, None]
    q_l = np.arange(128)[None, :]
    m1 = (q_l >= kv_l)
    maskd = np.concatenate([m1, m1], axis=1).astype(BF)  # [128, 256] 0/1

    def chunked(wT, nk):
        # [nk*128, M] -> [128, nk*M] kk-chunk-major
        m = wT.shape[1]
        return np.ascontiguousarray(
            wT.reshape(nk, 128, m).transpose(1, 0, 2).reshape(128, nk * m))

    perm = _rope_perm()
    in_maps = []
    for c in range(8):
        b, g = c // 2, c % 2
        rows = slice(g * QR, (g + 1) * QR)
        wq_g = Wq[rows, :][perm, :].T   # [D, QR]
        wk_g = Wk[rows, :][perm, :].T
        wv_g = Wv[rows, :].T
        xT = x[b].T                      # [D, S]
        x8 = np.ascontiguousarray(
            xT.reshape(8, 128, 4, 512).transpose(1, 2, 0, 3)
            .reshape(128, 4 * 8 * 512)).astype(BF)
        woT = Wo[:, rows].T              # [QR, D]
        wod = np.ascontiguousarray(
            woT.reshape(4, 128, D).transpose(1, 0, 2).reshape(128, 4 * D)
        ).astype(BF)
        in_maps.append({
            "x8": x8,
            "wq8": chunked(wq_g, 8).astype(BF),
            "wk8": chunked(wk_g, 8).astype(BF),
            "wv8": chunked(wv_g, 8).astype(BF),
            "wod": wod,
            "cos4": cos4,
            "sin4": sin4,
            "maskd": maskd,
        })
    return in_maps


def kernel(x, token_positions, Wq, Wk, Wv, Wo):
    global _COMPILED
    x = np.asarray(x, dtype=np.float32)
    token_positions = np.asarray(token_positions)
    Wq = np.asarray(Wq, dtype=np.float32)
    Wk = np.asarray(Wk, dtype=np.float32)
    Wv = np.asarray(Wv, dtype=np.float32)
    Wo = np.asarray(Wo, dtype=np.float32)

    if _COMPILED is None:
        _COMPILED = build_kernel()
    nc = _COMPILED

    in_maps = _prep_in_maps(x, token_positions, Wq, Wk, Wv, Wo)
    res = run_bass_kernel_spmd(nc, in_maps, core_ids=list(range(8)))

    out = np.empty((B, S, D), dtype=np.float32)
    for b in range(B):
        out[b] = (res.results[2 * b]["out"].astype(np.float32)
                  + res.results[2 * b + 1]["out"].astype(np.float32))
    return out


def time_device(inputs, n1=32, n2=128, repeats=2):
    """Async-pipelined device timing: enqueue N executions of the sharded
    PJRT call with device-resident inputs, block once.  The marginal
    (T(n2)-T(n1))/(n2-n1) cancels per-dispatch axon overhead and
    approximates per-execution hardware time.  Returns ns."""
    import time

    import jax
    from jax.sharding import Mesh, NamedSharding, PartitionSpec

    try:
        from jax.experimental.shard_map import shard_map
    except ImportError:
        from jax import shard_map

    from concourse import bass2jax

    global _COMPILED
    if _COMPILED is None:
        _COMPILED = build_kernel()
    nc = _COMPILED
    bass2jax.install_neuronx_cc_hook()

    in_maps = _prep_in_maps(
        np.asarray(inputs["x"], np.float32), np.asarray(inputs["token_positions"]),
        np.asarray(inputs["Wq"], np.float32), np.asarray(inputs["Wk"], np.float32),
        np.asarray(inputs["Wv"], np.float32), np.asarray(inputs["Wo"], np.float32))

    partition_name = (nc.partition_id_tensor.name
                      if nc.partition_id_tensor else None)
    in_names, out_names, out_avals, zero_outs = [], [], [], []
    for alloc in nc.m.functions[0].allocations:
        if not isinstance(alloc, mybir.MemoryLocationSet):
            continue
        name = alloc.memorylocations[0].name
        if alloc.kind == "ExternalInput":
            if name != partition_name:
                in_names.append(name)
        elif alloc.kind == "ExternalOutput":
            out_names.append(name)
            shape = tuple(alloc.tensor_shape)
            dtype = mybir.dt.np(alloc.dtype)
            out_avals.append(jax.core.ShapedArray(shape, dtype))
            zero_outs.append(np.zeros(shape, dtype))
    n_params = len(in_names)
    all_in_names = in_names + out_names
    if partition_name is not None:
        all_in_names = all_in_names + [partition_name]

    def _body(*args):
        operands = list(args)
        if partition_name is not None:
            operands.append(bass2jax.partition_id_tensor())
        outs = bass2jax._bass_exec_p.bind(
            *operands,
            out_avals=tuple(out_avals),
            in_names=tuple(all_in_names),
            out_names=tuple(out_names),
            lowering_input_output_aliases=(),
            sim_require_finite=True,
            sim_require_nnan=True,
            nc=nc,
        )
        return tuple(outs)

    n_cores = 8
    devices = jax.devices()[:n_cores]
    mesh = Mesh(np.asarray(devices), ("core",))
    spec = PartitionSpec("core")
    sharded = jax.jit(
        shard_map(_body, mesh=mesh,
                  in_specs=(spec,) * (n_params + len(out_names)),
                  out_specs=(spec,) * len(out_names), check_rep=False))
    sharding = NamedSharding(mesh, spec)
    dev_args = [
        jax.device_put(
            np.concatenate([np.asarray(in_maps[c][nm]) for c in range(n_cores)],
                           axis=0), sharding)
        for nm in in_names
    ] + [
        jax.device_put(
            np.zeros((n_cores * z.shape[0], *z.shape[1:]), z.dtype), sharding)
        for z in zero_outs
    ]

    jax.block_until_ready(sharded(*dev_args))

    def run_batch(n):
        t0 = time.perf_counter()
        outs = None
        for _ in range(n):
            outs = sharded(*dev_args)
        jax.block_until_ready(outs)
        return time.perf_counter() - t0

    best = None
    for _ in range(repeats):
        ta = run_batch(n1)
        tb = run_batch(n2)
        marg = (tb - ta) / (n2 - n1)
        best = marg if best is None else min(best, marg)
    return best * 1e9


# revision 28
# speedup vs baseline: 1.6155x; 1.6155x over previous
"""Trainium2 Bass kernel for causal multi-head attention with RoPE.

Problem: B=4, S=2048, D=1024, H=16, DK=64 dense transformer attention
(q/k/v projections -> interleaved RoPE on q,k -> causal softmax attention
-> output projection), fp32 inputs/outputs.

Sharding: 8 NeuronCores, core c handles batch b=c//2 and head-group
g=c%2 (8 of the 16 heads).  Each core computes a partial o_proj output
for its batch over its heads; the host sums the two partials per batch.

Kernel design (per core) — v1 (mixed precision + pipelined):
  - q/k/v projections in fp8e4 with DoubleRow perf mode (2 contraction
    chunks per matmul, 0.5 cyc/row): host delivers x and Wq/Wk/Wv in
    fp8e4, kk-chunk-major layouts so DoubleRow pair APs are strided views.
  - RoPE in bf16 on DVE (2x/4x packed modes) using host cos/sin tables;
    results DMA-permuted into head-contiguous bf16 qrh/krh tiles.
  - scores in bf16: S_ps[kv, q] = k_chunk @ qT, two heads per PE pass via
    tile_position row groups (K=64 each), both heads' scores in one
    [128, 1024] PSUM tile (two banks); ONE merged exp per chunk
    (strided [128, 2, N] access) -> pt bf16.
  - v stored bf16 with a per-head ones-column (65 cols/head) so attn@v
    also produces the softmax denominator row; attn@v in bf16.
  - software pipelining: scores/exp of chunk c+1 issue before attn@v of
    chunk c, so the PE never waits on the Act engine exp.
  - phase interleave: attention for q-group qg runs between projection
    blocks of sc=qg+1, overlapping projection PE work with attention
    Act/DVE work.
  - normalize: broadcast the denominator row l across 64 partitions with
    a K=1 ones matmul into a shared scratch PSUM bank, reciprocal +
    multiply on DVE -> normalized bf16 outT per head.
  - o_proj in bf16 accumulated in PSUM; outputs DMA'd as bf16, host sums
    the two per-batch partials in fp32.
"""

import sys

sys.path.insert(0, "/opt/trn_rl_repo")

from contextlib import ExitStack

import numpy as np

import concourse.bass as bass
import concourse.tile as tile
from concourse import bacc, mybir
from concourse.bass_utils import run_bass_kernel_spmd

B, S, D, H = 4, 2048, 1024, 16
DK = D // H          # 64
NHL = 8              # heads per core (local)
QR = NHL * DK        # 512 projected rows per core
NKC = S // 128       # 16 kv chunks
THETA = 10000.0

F32 = mybir.dt.float32
BF16 = mybir.dt.bfloat16
F8 = mybir.dt.float8e4
DR = mybir.MatmulPerfMode.DoubleRow

_COMPILED = None


def build_kernel(reps=1):
    nc = bacc.Bacc("TRN2", target_bir_lowering=False, debug=False,
                   enable_asserts=False)

    x8d = nc.dram_tensor("x8", [128, 4 * 8 * 512], BF16, kind="ExternalInput").ap()
    wq8 = nc.dram_tensor("wq8", [128, 8 * QR], BF16, kind="ExternalInput").ap()
    wk8 = nc.dram_tensor("wk8", [128, 8 * QR], BF16, kind="ExternalInput").ap()
    wv8 = nc.dram_tensor("wv8", [128, 8 * QR], BF16, kind="ExternalInput").ap()
    wod = nc.dram_tensor("wod", [128, 4 * D], BF16, kind="ExternalInput").ap()
    cos4 = nc.dram_tensor("cos4", [128, S], BF16, kind="ExternalInput").ap()
    sin4 = nc.dram_tensor("sin4", [128, S], BF16, kind="ExternalInput").ap()
    maskd = nc.dram_tensor("maskd", [128, 256], BF16, kind="ExternalInput").ap()
    out = nc.dram_tensor("out", [S, D], BF16, kind="ExternalOutput").ap()

    with tile.TileContext(nc) as tc, ExitStack() as ctx:
        persist = ctx.enter_context(tc.tile_pool(name="persist", bufs=1))
        # head-contiguous rope'd q/k: chunk hp holds heads (2hp, 2hp+1);
        # within a head: [even-lane j 0..31 ; odd-lane j 0..31]
        qrh = [persist.tile([128, S], BF16, tag=f"qrh{i}", name=f"qrh{i}")
               for i in range(4)]
        krh = [persist.tile([128, S], BF16, tag=f"krh{i}", name=f"krh{i}")
               for i in range(4)]
        # v natural layout, 65 cols per head (64 v + ones), all 16 s-tiles
        v_all = persist.tile([128, NKC * NHL * 65], BF16, tag="v_all")
        vsb = [v_all[:, i * NHL * 65:(i + 1) * NHL * 65] for i in range(NKC)]
        maskt = persist.tile([128, 256], BF16, tag="maskt")
        onest = persist.tile([65, 64], BF16, tag="onest")
        cost_all = persist.tile([128, S], BF16, tag="cost")
        sint_all = persist.tile([128, S], BF16, tag="sint")
        wq = persist.tile([128, 8 * QR], BF16, tag="wq")
        wk = persist.tile([128, 8 * QR], BF16, tag="wk")
        wv = persist.tile([128, 8 * QR], BF16, tag="wv")
        woh = persist.tile([128, 4 * D], BF16, tag="woh")

        nc.sync.dma_start(wq[:], wq8[:])
        nc.sync.dma_start(cost_all[:], cos4[:])
        nc.sync.dma_start(sint_all[:], sin4[:])
        nc.sync.dma_start(wv[:], wv8[:])
        nc.sync.dma_start(wk[:], wk8[:])
        nc.scalar.dma_start(maskt[:], maskd[:])
        m3 = maskt[:].rearrange("p (two n) -> p two n", two=2)
        nc.scalar.dma_start(woh[:], wod[:])
        nc.vector.memset(onest[:], 1.0)
        # ones column (col 64 of each head's 65-col block), all kv tiles
        v3 = v_all[:].rearrange("p (n c) -> p n c", c=65)
        nc.gpsimd.memset(v3[:, :, 64:65], 1.0)

        wq3 = wq[:].rearrange("p (k q) -> p k q", k=8)
        wk3 = wk[:].rearrange("p (k q) -> p k q", k=8)
        wv3 = wv[:].rearrange("p (k q) -> p k q", k=8)

        xpool = ctx.enter_context(tc.tile_pool(name="xp", bufs=2))
        stg = ctx.enter_context(tc.tile_pool(name="stg", bufs=2))
        ppool = ctx.enter_context(tc.tile_pool(name="pt", bufs=6))
        otn = ctx.enter_context(tc.tile_pool(name="otn", bufs=6))
        # PSUM budget (8 banks): scratch 2 + sp 2x2 + O 2
        ps_x = ctx.enter_context(
            tc.tile_pool(name="ps_x", bufs=2, space="PSUM"))
        ps_s = ctx.enter_context(
            tc.tile_pool(name="ps_s", bufs=2, space="PSUM"))
        ps_o = ctx.enter_context(
            tc.tile_pool(name="ps_o", bufs=1, space="PSUM"))

        def proj_chunk(w3, xt3, m, names):
            """One m-chunk [128, 512] of a q/k projection, bf16."""
            ps = ps_x.tile([128, 512], F32, tag="scr", name="pps")
            for t in range(8):
                nc.tensor.matmul(
                    ps[:],
                    w3[:, t, m * 128:(m + 1) * 128],
                    xt3[:, t, :],
                    start=(t == 0), stop=(t == 7))
            qs = stg.tile([128, 512], BF16, tag="qps", bufs=8,
                          name=f"{names}{m}")
            nc.vector.tensor_copy(qs[:], ps[:])
            return qs

        def project(w3, xt3, names):
            return [proj_chunk(w3, xt3, m, names) for m in range(4)]

        def vproj_chunk(xt3, sc, st):
            vp = ps_x.tile([128, 512], F32, tag="scr", name="vp")
            for t in range(8):
                nc.tensor.matmul(
                    vp[:],
                    xt3[:, t, st * 128:(st + 1) * 128],
                    wv3[:, t, :],
                    start=(t == 0), stop=(t == 7))
            vdst = vsb[sc * 4 + st][:].rearrange(
                "p (h c) -> p h c", c=65)[:, :, 0:64]
            vsrc = vp[:].rearrange("p (h c) -> p h c", c=64)
            nc.vector.tensor_copy(vdst, vsrc)

        def rope(sb, dst, s0, qk):
            # chunks (0,2) even/odd of heads 0-3, (1,3) heads 4-7
            costc = cost_all[:, s0:s0 + 512]
            sintc = sint_all[:, s0:s0 + 512]
            for me, mo in ((0, 2), (1, 3)):
                hbase = 0 if me == 0 else 4
                te = stg.tile([128, 512], BF16, tag="tmp", bufs=4)
                to = stg.tile([128, 512], BF16, tag="tmp", bufs=4)
                nc.vector.tensor_mul(te[:], sb[me][:], costc)
                nc.gpsimd.tensor_mul(to[:], sb[mo][:], sintc)
                qre = stg.tile([128, 512], BF16, tag="qr", bufs=4)
                nc.vector.tensor_sub(qre[:], te[:], to[:])
                te2 = stg.tile([128, 512], BF16, tag="tmp", bufs=4)
                to2 = stg.tile([128, 512], BF16, tag="tmp", bufs=4)
                nc.gpsimd.tensor_mul(te2[:], sb[mo][:], costc)
                nc.vector.tensor_mul(to2[:], sb[me][:], sintc)
                qro = stg.tile([128, 512], BF16, tag="qr", bufs=4)
                nc.vector.tensor_add(qro[:], te2[:], to2[:])
                # permute into head-contiguous chunks via DMA
                for hl in range(4):
                    h = hbase + hl
                    hp, h01 = h // 2, h % 2
                    nc.sync.dma_start(
                        dst[hp][64 * h01: 64 * h01 + 32, s0:s0 + 512],
                        qre[32 * hl: 32 * hl + 32, :])
                    nc.sync.dma_start(
                        dst[hp][64 * h01 + 32: 64 * h01 + 64, s0:s0 + 512],
                        qro[32 * hl: 32 * hl + 32, :])

        def attention(qg, fill=None):
            q0 = qg * 512
            nchunks = 4 * qg + 4
            otn_tiles = [None] * 4
            for hp in range(4):
                O = [ps_o.tile([65, 512], F32, tag=f"O{h01}", name="O")
                     for h01 in range(2)]
                pend = None

                def emit_av(ent):
                    c, pt, qoff, N = ent
                    for h01 in range(2):
                        h = 2 * hp + h01
                        nc.tensor.matmul(
                            O[h01][:, qoff:qoff + N],
                            vsb[c][:, 65 * h: 65 * h + 65],
                            pt[:, 512 * h01: 512 * h01 + N],
                            start=(c == 0), stop=(c == nchunks - 1))

                for c in range(nchunks):
                    cmod = c - 4 * qg
                    qoff = 128 * cmod if cmod >= 0 else 0
                    N = 512 - qoff
                    sp = ps_s.tile([128, 1024], F32, tag="S")
                    for h01 in range(2):
                        base = 64 * h01
                        nc.tensor.matmul(
                            sp[:, 512 * h01: 512 * h01 + N],
                            krh[hp][base:base + 64, c * 128:(c + 1) * 128],
                            qrh[hp][base:base + 64, q0 + qoff:q0 + qoff + N],
                            start=True, stop=True,
                            tile_position=(base, 0))
                    pt = ppool.tile([128, 1024], BF16, tag="pt", bufs=6)
                    if N == 512:
                        nc.scalar.activation(
                            pt[:], sp[:],
                            mybir.ActivationFunctionType.Exp, scale=0.125)
                    else:
                        sp3 = sp[:].rearrange("p (two n) -> p two n", two=2)
                        pt3 = pt[:].rearrange("p (two n) -> p two n", two=2)
                        nc.scalar.activation(
                            pt3[:, :, 0:N], sp3[:, :, 0:N],
                            mybir.ActivationFunctionType.Exp, scale=0.125)
                    if cmod >= 0:
                        # causal mask: zero upper triangle post-exp
                        pt3 = pt[:].rearrange("p (two n) -> p two n", two=2)
                        nc.gpsimd.tensor_mul(pt3[:, :, 0:128],
                                             pt3[:, :, 0:128], m3[:])
                    if pend is not None:
                        emit_av(pend)
                    pend = (c, pt, qoff, N)
                emit_av(pend)

                pair = otn.tile([128, 512], BF16, tag="pair", bufs=6,
                                name="pair")
                for h01 in range(2):
                    lsb = stg.tile([65, 512], BF16, tag="lsb", bufs=2)
                    nc.vector.tensor_copy(lsb[64:65, :], O[h01][64:65, :])
                    rbp = ps_x.tile([128, 512], F32, tag="scr", name="rbp")
                    nc.tensor.matmul(rbp[0:64, :],
                                     onest[64:65, 0:64],
                                     lsb[64:65, :],
                                     start=True, stop=True)
                    rlb = stg.tile([64, 512], F32, tag="rlb", bufs=2)
                    nc.vector.reciprocal(rlb[:], rbp[0:64, :])
                    if h01 == 0:
                        nc.vector.tensor_mul(pair[0:64, :],
                                             O[h01][0:64, :], rlb[:])
                    else:
                        ot = stg.tile([64, 512], BF16, tag="ot", bufs=2)
                        nc.vector.tensor_mul(ot[:], O[h01][0:64, :],
                                             rlb[:])
                        nc.gpsimd.dma_start(pair[64:128, :], ot[:])
                otn_tiles[hp] = pair
                if fill is not None:
                    for thunk in fill[hp]:
                        thunk()

            # o_proj for this q-group
            for qt in range(4):
                qtile = qg * 4 + qt
                osb = stg.tile([128, 1024], BF16, tag="osb", bufs=3)
                for oh in range(2):
                    f = ps_x.tile([128, 512], F32, tag="scr", name="F")
                    for p in range(4):
                        nc.tensor.matmul(
                            f[:],
                            otn_tiles[p][:, qt * 128:(qt + 1) * 128],
                            woh[:, p * D + oh * 512: p * D + oh * 512 + 512],
                            start=(p == 0), stop=(p == 3))
                    nc.vector.tensor_copy(osb[:, oh * 512:(oh + 1) * 512],
                                          f[:])
                nc.sync.dma_start(
                    out[qtile * 128:(qtile + 1) * 128, :], osb[:])

        for _rep in range(reps):
            xts = {}

            def load_xt(sc):
                xt = xpool.tile([128, 8 * 512], BF16, tag="xt")
                nc.sync.dma_start(xt[:], x8d[:, sc * 4096:(sc + 1) * 4096])
                xts[sc] = xt[:].rearrange("p (k s) -> p k s", k=8)

            def proj_fill(sc):
                """Projection work for block sc as 4 thunk-lists, consumed
                at the hp boundaries of the preceding attention call."""
                s0 = sc * 512
                xt3 = xts[sc]
                qsb = []
                ksb = []

                def qm(m):
                    return lambda: qsb.append(proj_chunk(wq3, xt3, m, "q"))

                def km(m):
                    return lambda: ksb.append(proj_chunk(wk3, xt3, m, "k"))

                def vu(st):
                    return lambda: vproj_chunk(xt3, sc, st)

                def rq():
                    rope(qsb, qrh, s0, "q")

                def rk():
                    rope(ksb, krh, s0, "k")

                return [
                    [qm(0), qm(1)],
                    [qm(2), qm(3), vu(0)],
                    [rq, vu(1), vu(2), km(0)],
                    [vu(3), km(1), km(2), km(3), rk],
                ]

            load_xt(0)
            load_xt(1)
            # prologue: full projection of block 0
            s0 = 0
            xt3 = xts[0]
            qsb = project(wq3, xt3, "q")
            for st in range(4):
                vproj_chunk(xt3, 0, st)
            rope(qsb, qrh, s0, "q")
            ksb = project(wk3, xt3, "k")
            load_xt(2)
            rope(ksb, krh, s0, "k")

            for qg in range(4):
                sc = qg + 1
                if sc < 4:
                    if sc == 2:
                        load_xt(3)
                    attention(qg, fill=proj_fill(sc))
                else:
                    attention(qg)

    nc.compile()
    return nc


def _rope_perm():
    """Row permutation for Wq/Wk per-core slices: 4 chunks of 128 =
    (heads 0-3 even, heads 4-7 even, heads 0-3 odd, heads 4-7 odd)."""
    perm = []
    for half in (0, 1):
        for hblk in range(2):
            for h in range(4 * hblk, 4 * hblk + 4):
                for j in range(32):
                    perm.append(h * 64 + 2 * j + half)
    return np.array(perm)


def _prep_in_maps(x, token_positions, Wq, Wk, Wv, Wo):
    BF = mybir.dt.np(BF16)
    half = DK // 2
    freqs = (1.0 / (THETA ** (2.0 * np.arange(half, dtype=np.float32) / DK)))
    angles = token_positions.astype(np.float32)[:, None] * freqs[None, :]
    cos = np.cos(angles).astype(np.float32).T    # [32, S]
    sin = np.sin(angles).astype(np.float32).T
    cos4 = np.ascontiguousarray(np.tile(cos, (4, 1))).astype(BF)  # [128, S]
    sin4 = np.ascontiguousarray(np.tile(sin, (4, 1))).astype(BF)

    kv_l = np.arange(128)[:, None]
    q_l = np.arange(128)[None, :]
    m1 = (q_l >= kv_l)
    maskd = np.concatenate([m1, m1], axis=1).astype(BF)  # [128, 256] 0/1

    def chunked(wT, nk):
        # [nk*128, M] -> [128, nk*M] kk-chunk-major
        m = wT.shape[1]
        return np.ascontiguousarray(
            wT.reshape(nk, 128, m).transpose(1, 0, 2).reshape(128, nk * m))

    perm = _rope_perm()
    in_maps = []
    for c in range(8):
        b, g = c // 2, c % 2
        rows = slice(g * QR, (g + 1) * QR)
        wq_g = Wq[rows, :][perm, :].T   # [D, QR]
        wk_g = Wk[rows, :][perm, :].T
        wv_g = Wv[rows, :].T
        xT = x[b].T                      # [D, S]
        x8 = np.ascontiguousarray(
            xT.reshape(8, 128, 4, 512).transpose(1, 2, 0, 3)
            .reshape(128, 4 * 8 * 512)).astype(BF)
        woT = Wo[:, rows].T              # [QR, D]
        wod = np.ascontiguousarray(
            woT.reshape(4, 128, D).transpose(1, 0, 2).reshape(128, 4 * D)
        ).astype(BF)
        in_maps.append({
            "x8": x8,
            "wq8": chunked(wq_g, 8).astype(BF),
            "wk8": chunked(wk_g, 8).astype(BF),
            "wv8": chunked(wv_g, 8).astype(BF),
            "wod": wod,
            "cos4": cos4,
            "sin4": sin4,
            "maskd": maskd,
        })
    return in_maps


def kernel(x, token_positions, Wq, Wk, Wv, Wo):
    global _COMPILED
    x = np.asarray(x, dtype=np.float32)
    token_positions = np.asarray(token_positions)
    Wq = np.asarray(Wq, dtype=np.float32)
    Wk = np.asarray(Wk, dtype=np.float32)
    Wv = np.asarray(Wv, dtype=np.float32)
    Wo = np.asarray(Wo, dtype=np.float32)

    if _COMPILED is None:
        _COMPILED = build_kernel()
    nc = _COMPILED

    in_maps = _prep_in_maps(x, token_positions, Wq, Wk, Wv, Wo)
    res = run_bass_kernel_spmd(nc, in_maps, core_ids=list(range(8)))

    out = np.empty((B, S, D), dtype=np.float32)
    for b in range(B):
        out[b] = (res.results[2 * b]["out"].astype(np.float32)
                  + res.results[2 * b + 1]["out"].astype(np.float32))
    return out


def time_device(inputs, n1=32, n2=128, repeats=2):
    """Async-pipelined device timing: enqueue N executions of the sharded
    PJRT call with device-resident inputs, block once.  The marginal
    (T(n2)-T(n1))/(n2-n1) cancels per-dispatch axon overhead and
    approximates per-execution hardware time.  Returns ns."""
    import time

    import jax
    from jax.sharding import Mesh, NamedSharding, PartitionSpec

    try:
        from jax.experimental.shard_map import shard_map
    except ImportError:
        from jax import shard_map

    from concourse import bass2jax

    global _COMPILED
    if _COMPILED is None:
        _COMPILED = build_kernel()
    nc = _COMPILED
    bass2jax.install_neuronx_cc_hook()

    in_maps = _prep_in_maps(
        np.asarray(inputs["x"], np.float32), np.asarray(inputs["token_positions"]),
        np.asarray(inputs["Wq"], np.float32), np.asarray(inputs["Wk"], np.float32),
        np.asarray(inputs["Wv"], np.float32), np.asarray(inputs["Wo"], np.float32))

    partition_name = (nc.partition_id_tensor.name
                      if nc.partition_id_tensor else None)
    in_names, out_names, out_avals, zero_outs = [], [], [], []
    for alloc in nc.m.functions[0].allocations:
        if not isinstance(alloc, mybir.MemoryLocationSet):
            continue
        name = alloc.memorylocations[0].name
        if alloc.kind == "ExternalInput":
            if name != partition_name:
                in_names.append(name)
        elif alloc.kind == "ExternalOutput":
            out_names.append(name)
            shape = tuple(alloc.tensor_shape)
            dtype = mybir.dt.np(alloc.dtype)
            out_avals.append(jax.core.ShapedArray(shape, dtype))
            zero_outs.append(np.zeros(shape, dtype))
    n_params = len(in_names)
    all_in_names = in_names + out_names
    if partition_name is not None:
        all_in_names = all_in_names + [partition_name]

    def _body(*args):
        operands = list(args)
        if partition_name is not None:
            operands.append(bass2jax.partition_id_tensor())
        outs = bass2jax._bass_exec_p.bind(
            *operands,
            out_avals=tuple(out_avals),
            in_names=tuple(all_in_names),
            out_names=tuple(out_names),
            lowering_input_output_aliases=(),
            sim_require_finite=True,
            sim_require_nnan=True,
            nc=nc,
        )
        return tuple(outs)

    n_cores = 8
    devices = jax.devices()[:n_cores]
    mesh = Mesh(np.asarray(devices), ("core",))
    spec = PartitionSpec("core")
    sharded = jax.jit(
        shard_map(_body, mesh=mesh,
                  in_specs=(spec,) * (n_params + len(out_names)),
                  out_specs=(spec,) * len(out_names), check_rep=False))
    sharding = NamedSharding(mesh, spec)
    dev_args = [
        jax.device_put(
            np.concatenate([np.asarray(in_maps[c][nm]) for c in range(n_cores)],
                           axis=0), sharding)
        for nm in in_names
    ] + [
        jax.device_put(
            np.zeros((n_cores * z.shape[0], *z.shape[1:]), z.dtype), sharding)
        for z in zero_outs
    ]

    jax.block_until_ready(sharded(*dev_args))

    def run_batch(n):
        t0 = time.perf_counter()
        outs = None
        for _ in range(n):
            outs = sharded(*dev_args)
        jax.block_until_ready(outs)
        return time.perf_counter() - t0

    best = None
    for _ in range(repeats):
        ta = run_batch(n1)
        tb = run_batch(n2)
        marg = (tb - ta) / (n2 - n1)
        best = marg if best is None else min(best, marg)
    return best * 1e9
